# revision 10
# baseline (speedup 1.0000x reference)
"""TRN2 Bass kernel for CompressedCausalAttention.

Problem: x,pe [S=1024, B=8, C=768]; qkv = (x+pe) @ W_qkv + b_qkv; 12 heads of 64;
causal softmax attention; out proj. Sharding: data-parallel over batch, one
batch element per NeuronCore (8 cores).

Per-core dataflow (all channel-major to avoid transposing activations twice):
  xpeT [C, S]   = PE-transpose(x+pe)
  qkT  [2C', S] = W_qk^T @ xpeT (+ b_qk per-partition during PSUM->SBUF copy)
  V    [S, C]   = xpeT^T @ W_v  (stored per t-block with a ones column per head
                                 -> PV matmul also produces softmax denominators)
  per head h, per t-block Ti:
     scoresT[t, s] = kT_h[:,Ti]^T @ qT_h  (only s >= 128*Ti; causal)
     += -1e30 upper-tri mask on the diagonal block
     pT = exp(0.125 * scoresT)            (ACT, PSUM->SBUF)
     pv[cc+1, s] += V'_h[Ti]^T @ pT       (row 64 = denominator)
  normalize: rdenom = 1/denom; broadcast over head rows via 0/1 E-matrix matmul;
  pvT *= bcast.  y [S, C] = pvT^T @ W_o + b_out_eff (rank-1 ones matmul).
  b_out_eff folds the V bias through W_out on the host (softmax rows sum to 1).

Matmuls run in fp32r (fp32 truncated to ~11-bit mantissa at the PE, fp32
accumulate) for 4x PE throughput vs true fp32; set BASSK_F32=1 for full fp32.
"""
import os
import numpy as np

import concourse.bass as bass
import concourse.bacc as bacc
import concourse.mybir as mybir
import concourse.tile as tile
from concourse.bass_utils import run_bass_kernel_spmd

S, B, C, H = 1024, 8, 768, 12
CC = C // H            # 64
NS = S // 128          # 8 s/t blocks
NK = C // 128          # 6 contraction chunks
NM = 2 * C // 128      # 12 q+k M-tiles
F32 = mybir.dt.float32
USE_F32 = os.environ.get("BASSK_F32", "0") == "1"
MDT = F32 if USE_F32 else mybir.dt.float32r
AF = mybir.ActivationFunctionType
ALU = mybir.AluOpType

_CACHE = {}


def _build():
    nc = bacc.Bacc("TRN2", target_bir_lowering=False, debug=False)

    Xb = nc.dram_tensor("xb", [S, C], F32, kind="ExternalInput")
    PEb = nc.dram_tensor("peb", [S, C], F32, kind="ExternalInput")
    Wqk = nc.dram_tensor("wqk", [C, 2 * C], MDT, kind="ExternalInput")
    Wv = nc.dram_tensor("wv", [C, C], MDT, kind="ExternalInput")
    Wo = nc.dram_tensor("wo", [C, C], MDT, kind="ExternalInput")
    Bqk = nc.dram_tensor("bqk", [128, NM], F32, kind="ExternalInput")
    Beff = nc.dram_tensor("beff", [1, C], MDT, kind="ExternalInput")
    Ident = nc.dram_tensor("ident", [128, 128], F32, kind="ExternalInput")
    MaskNeg = nc.dram_tensor("maskneg", [128, 128], F32, kind="ExternalInput")
    Emat = nc.dram_tensor("emat", [H, C], MDT, kind="ExternalInput")
    Ones1 = nc.dram_tensor("ones1", [1, 128], MDT, kind="ExternalInput")
    OnesV = nc.dram_tensor("onesv", [128, H], MDT, kind="ExternalInput")
    Y = nc.dram_tensor("y", [S, C], F32, kind="ExternalOutput")

    from contextlib import ExitStack
    with ExitStack() as _es:
        tc = _es.enter_context(tile.TileContext(nc))
        _p = lambda **kw: _es.enter_context(tc.tile_pool(**kw))
        cst = _p(name="cst", bufs=1)
        ld = _p(name="ld", bufs=2)
        xpe = _p(name="xpe", bufs=2)
        xpeT_p = _p(name="xpeT", bufs=NK)
        wqk_p = _p(name="wqk", bufs=NK)
        wvo_p = _p(name="wvo", bufs=NK)
        qkT_p = _p(name="qkT", bufs=4)
        vx_p = _p(name="vx", bufs=NS)
        pT_p = _p(name="pT", bufs=2)
        pvT_p = _p(name="pvT", bufs=NK)
        ysb_p = _p(name="ysb", bufs=2)
        dstage_p = _p(name="dstage", bufs=2)
        ps = _p(name="ps", bufs=3, space="PSUM")
        pvps = _p(name="pvps", bufs=1, space="PSUM")
        tps = vps = qkps = scps = bcps = yps = ps
        if True:
            ident_sb = cst.tile([128, 128], F32, tag="ident")
            nc.sync.dma_start(ident_sb[:], Ident.ap())
            mask_sb = cst.tile([128, 128], F32, tag="mask")
            nc.sync.dma_start(mask_sb[:], MaskNeg.ap())
            bqk_sb = cst.tile([128, NM], F32, tag="bqk")
            nc.sync.dma_start(bqk_sb[:], Bqk.ap())
            beff_sb = cst.tile([1, C], MDT, tag="beff")
            nc.sync.dma_start(beff_sb[:], Beff.ap())
            emat_sb = cst.tile([H, C], MDT, tag="emat")
            nc.sync.dma_start(emat_sb[:], Emat.ap())
            ones_sb = cst.tile([1, 128], MDT, tag="ones")
            nc.sync.dma_start(ones_sb[:], Ones1.ap())
            den_sb = cst.tile([H, S], F32, tag="den")
            rden_sb = cst.tile([H, S], MDT, tag="rden")

            # Weights (wvo pool serves W_v first, then W_o reuses the slots)
            wqk_t = []
            for k in range(NK):
                w = wqk_p.tile([128, 2 * C], MDT, tag="wqk", name=f"wqk{k}")
                nc.sync.dma_start(w[:], Wqk.ap()[k * 128:(k + 1) * 128, :])
                wqk_t.append(w)
            wv_t = []
            for k in range(NK):
                w = wvo_p.tile([128, C], MDT, tag="wvo", name=f"wv{k}")
                nc.sync.dma_start(w[:], Wv.ap()[k * 128:(k + 1) * 128, :])
                wv_t.append(w)

            # ---- Phase A: xpeT = (x + pe)^T  [C, S] as NK tiles [128, S]
            xpeT = [xpeT_p.tile([128, S], MDT, tag="xpeT", name=f"xpeT{j}") for j in range(NK)]
            for i in range(NS):
                xt = ld.tile([128, C], F32, tag="ldx")
                pt = ld.tile([128, C], F32, tag="ldp")
                nc.sync.dma_start(xt[:], Xb.ap()[i * 128:(i + 1) * 128, :])
                nc.sync.dma_start(pt[:], PEb.ap()[i * 128:(i + 1) * 128, :])
                xp = xpe.tile([128, C], F32, tag="xpe")
                nc.vector.tensor_tensor(xp[:], xt[:], pt[:], ALU.add)
                for j in range(NK):
                    tp = tps.tile([128, 128], F32, tag="ps")
                    nc.tensor.transpose(tp[:], xp[:, j * 128:(j + 1) * 128], ident_sb[:])
                    nc.any.tensor_copy(xpeT[j][:, i * 128:(i + 1) * 128], tp[:])

            # ---- Phase A2: V with ones column per head: vx[Ti] [128, 780]
            vx = []
            for si in range(NS):
                vp = vps.tile([128, C], F32, tag="ps")
                for (c0, c1) in ((0, 512), (512, C)):
                    for k in range(NK):
                        nc.tensor.matmul(
                            vp[:, c0:c1],
                            xpeT[k][:, si * 128:(si + 1) * 128],
                            wv_t[k][:, c0:c1],
                            start=(k == 0), stop=(k == NK - 1),
                        )
                v = vx_p.tile([128, H * (CC + 1)], MDT, tag="vx", name=f"vx{si}")
                v3 = v[:].rearrange("p (h c) -> p h c", h=H)
                nc.any.tensor_copy(
                    v3[:, :, 0:CC], vp[:].rearrange("p (h c) -> p h c", h=H)
                )
                nc.sync.dma_start(v3[:, :, CC:CC + 1], OnesV.ap()[:, :, None])
                vx.append(v)

            # ---- Phases B+C: per pass p: qkT tiles (p, 6+p) then heads 2p, 2p+1
            pvT = [pvT_p.tile([128, S], MDT, tag="pvT", name=f"pvT{j}") for j in range(NK)]
            for p in range(6):
                qk = {}
                for mm in (p, 6 + p):
                    ps_ = qkps.tile([128, S], F32, tag="ps")
                    for n2 in (0, 1):
                        for k in range(NK):
                            nc.tensor.matmul(
                                ps_[:, n2 * 512:(n2 + 1) * 512],
                                wqk_t[k][:, mm * 128:(mm + 1) * 128],
                                xpeT[k][:, n2 * 512:(n2 + 1) * 512],
                                start=(k == 0), stop=(k == NK - 1),
                            )
                    t = qkT_p.tile([128, S], MDT, tag="qkT", name=f"qkT{mm}")
                    nc.scalar.activation(
                        t[:], ps_[:], AF.Identity, bias=bqk_sb[:, mm:mm + 1]
                    )
                    qk[mm] = t
                for h in (2 * p, 2 * p + 1):
                    r0 = (h % 2) * CC
                    qt, kt = qk[p], qk[6 + p]
                    pv = pvps.tile([CC + 1, S], F32, tag="pvps")
                    for Ti in range(NS):
                        s0 = Ti * 128
                        chunks = [(s0, 512), (512, S)] if s0 < 512 else [(s0, S)]
                        sc = scps.tile([128, S], F32, tag="ps")
                        for (a0, a1) in chunks:
                            nc.tensor.matmul(
                                sc[:, a0:a1],
                                kt[r0:r0 + CC, s0:s0 + 128],
                                qt[r0:r0 + CC, a0:a1],
                                start=True, stop=True,
                            )
                        nc.vector.tensor_tensor(
                            sc[:, s0:s0 + 128], sc[:, s0:s0 + 128], mask_sb[:], ALU.add
                        )
                        pt = pT_p.tile([128, S], MDT, tag="pT")
                        nc.scalar.activation(
                            pt[:, s0:S], sc[:, s0:S], AF.Exp, scale=1.0 / np.sqrt(CC)
                        )
                        v3 = vx[Ti][:].rearrange("p (h c) -> p h c", h=H)
                        for (a0, a1) in chunks:
                            # one accumulation group per psum bank: bank0
                            # (cols <512) last written at Ti=3, bank1 at Ti=7
                            last_ti = 3 if a1 <= 512 else NS - 1
                            nc.tensor.matmul(
                                pv[:, a0:a1],
                                v3[:, h, :],
                                pt[:, a0:a1],
                                start=(Ti == 0), stop=(Ti == last_ti),
                            )
                    dst = dstage_p.tile([1, S], F32, tag="dstage", name=f"dst{h}")
                    nc.scalar.copy(dst[:], pv[CC:CC + 1, :])
                    nc.sync.dma_start(den_sb[h:h + 1, :], dst[:])
                    nc.any.tensor_copy(pvT[h // 2][r0:r0 + CC, :], pv[0:CC, :])

            # ---- Phase D: normalize + output projection
            with nc.allow_low_precision(reason="fp32r rounding of softmax scale"):
                nc.vector.reciprocal(rden_sb[:], den_sb[:])
            for j in range(NK):
                bc = bcps.tile([128, S], F32, tag="ps")
                for n2 in (0, 1):
                    nc.tensor.matmul(
                        bc[:, n2 * 512:(n2 + 1) * 512],
                        emat_sb[:, j * 128:(j + 1) * 128],
                        rden_sb[:, n2 * 512:(n2 + 1) * 512],
                        start=True, stop=True,
                    )
                nc.vector.tensor_tensor(pvT[j][:], pvT[j][:], bc[:], ALU.mult)

            wo_t = []
            for k in range(NK):
                w = wvo_p.tile([128, C], MDT, tag="wvo", name=f"wo{k}")
                nc.sync.dma_start(w[:], Wo.ap()[k * 128:(k + 1) * 128, :])
                wo_t.append(w)
            for si in range(NS):
                yp = yps.tile([128, C], F32, tag="ps")
                for (c0, c1) in ((0, 512), (512, C)):
                    for k in range(NK):
                        nc.tensor.matmul(
                            yp[:, c0:c1],
                            pvT[k][:, si * 128:(si + 1) * 128],
                            wo_t[k][:, c0:c1],
                            start=(k == 0), stop=False,
                        )
                    nc.tensor.matmul(
                        yp[:, c0:c1], ones_sb[:], beff_sb[:, c0:c1],
                        start=False, stop=True,
                    )
                ty = ysb_p.tile([128, C], F32, tag="ysb")
                nc.any.tensor_copy(ty[:], yp[:])
                nc.sync.dma_start(Y.ap()[si * 128:(si + 1) * 128, :], ty[:])

    nc.compile()
    return nc


def _round_fp22(a):
    """Round to nearest fp22 (11-bit mantissa) so the PE's truncation is exact."""
    b = a.astype(np.float32).view(np.uint32)
    b = (b + 0x800) & np.uint32(0xFFFFF000)
    return b.view(np.float32)


def _prep(inputs):
    x = np.asarray(inputs["x"], np.float32)
    pe = np.asarray(inputs["pe"], np.float32)
    W_qkv = np.asarray(inputs["W_qkv"], np.float32)
    b_qkv = np.asarray(inputs["b_qkv"], np.float32)
    W_out = np.asarray(inputs["W_out"], np.float32)
    b_out = np.asarray(inputs["b_out"], np.float32)

    rnd = (lambda a: a) if USE_F32 else _round_fp22
    wqk = rnd(np.ascontiguousarray(W_qkv[:, :2 * C]))
    wv = rnd(np.ascontiguousarray(W_qkv[:, 2 * C:]))
    wo = rnd(np.ascontiguousarray(W_out))
    bqk = np.ascontiguousarray(b_qkv[:2 * C].reshape(NM, 128).T)
    beff = rnd((b_qkv[2 * C:] @ W_out + b_out).reshape(1, C).astype(np.float32))
    ident = np.eye(128, dtype=np.float32)
    t = np.arange(128)
    maskneg = np.where(t[:, None] > t[None, :], np.float32(-1e30), np.float32(0.0))
    emat = np.zeros((H, C), np.float32)
    for h in range(H):
        emat[h, h * CC:(h + 1) * CC] = 1.0
    ones1 = np.ones((1, 128), np.float32)

    common = dict(wqk=wqk, wv=wv, wo=wo, bqk=bqk, beff=beff, ident=ident,
                  maskneg=maskneg.astype(np.float32), emat=emat, ones1=ones1,
                  onesv=np.ones((128, H), np.float32))
    in_maps = []
    for b in range(B):
        m = dict(common)
        m["xb"] = np.ascontiguousarray(x[:, b, :])
        m["peb"] = np.ascontiguousarray(pe[:, b, :])
        in_maps.append(m)
    return in_maps


def _run(inputs, trace=False):
    if "nc" not in _CACHE:
        _CACHE["nc"] = _build()
    nc = _CACHE["nc"]
    in_maps = _prep(inputs)
    res = run_bass_kernel_spmd(nc, in_maps, core_ids=list(range(B)), trace=trace)
    out = np.empty((S, B, C), np.float32)
    for b in range(B):
        out[:, b, :] = res.results[b]["y"]
    return out, res


def kernel(**inputs):
    out, _ = _run(inputs, trace=False)
    return out


# revision 18
# speedup vs baseline: 499.8178x; 499.8178x over previous
"""TRN2 Bass kernel for CompressedCausalAttention.

Problem: x,pe [S=1024, B=8, C=768]; qkv = (x+pe) @ W_qkv + b_qkv; 12 heads of 64;
causal softmax attention; out proj. Sharding: data-parallel over batch, one
batch element per NeuronCore (8 cores).

Per-core dataflow (channel-major to avoid transposing activations twice):
  xpeT [C, S]   = PE-transpose(x+pe)
  qkT  [2C', S] = W_qk^T @ xpeT (+ b_qk per-partition during PSUM->SBUF copy)
  V    [S, C]   = xpeT^T @ W_v  (stored per t-block with a ones column per head
                                 -> PV matmul also produces softmax denominators)
  per head h, per t-block Ti (both heads of a pass interleaved so their K=64
  QK matmuls occupy disjoint PE row halves and run concurrently):
     scoresT[t, s] = kT_h[:,Ti]^T @ qT_h   (only s >= 128*Ti; causal)
     pT = exp(0.125 * scoresT)             (ACT, PSUM->SBUF)
     diag block *= upper-tri 0/1 mask      (GPSIMD; zero masked probs)
     pv[cc+1, s] += V'_h[Ti]^T @ pT        (row 64 = denominator)
  normalize: rdenom = 1/denom; broadcast over head rows via 0/1 E-matrix matmul;
  pvT *= bcast.  y [S, C] = pvT^T @ W_o + b_out_eff (rank-1 ones matmul).
  b_out_eff folds the V bias through W_out on the host (softmax rows sum to 1).

Matmuls run in fp32r (fp32 truncated to ~11-bit mantissa at the PE, fp32
accumulate) for 4x PE throughput vs true fp32; set BASSK_F32=1 for full fp32.
"""
import os
import numpy as np

import concourse.bass as bass
import concourse.bacc as bacc
import concourse.mybir as mybir
import concourse.tile as tile
from concourse.bass_utils import run_bass_kernel_spmd

S, B, C, H = 1024, 8, 768, 12
CC = C // H            # 64
NS = S // 128          # 8 s/t blocks
NK = C // 128          # 6 contraction chunks
NM = 2 * C // 128      # 12 q+k M-tiles
F32 = mybir.dt.float32
USE_F32 = os.environ.get("BASSK_F32", "0") == "1"
REPEAT = int(os.environ.get("BASSK_REPEAT", "1"))
MDT = F32 if USE_F32 else mybir.dt.float32r
AF = mybir.ActivationFunctionType
ALU = mybir.AluOpType

_CACHE = {}


def _build():
    nc = bacc.Bacc("TRN2", target_bir_lowering=False, debug=False)

    Xb = nc.dram_tensor("xb", [S, C], F32, kind="ExternalInput")
    PEb = nc.dram_tensor("peb", [S, C], F32, kind="ExternalInput")
    Wqk = nc.dram_tensor("wqk", [C, 2 * C], MDT, kind="ExternalInput")
    Wv = nc.dram_tensor("wv", [C, C], MDT, kind="ExternalInput")
    Wo = nc.dram_tensor("wo", [C, C], MDT, kind="ExternalInput")
    Bqk = nc.dram_tensor("bqk", [128, NM], F32, kind="ExternalInput")
    Beff = nc.dram_tensor("beff", [1, C], MDT, kind="ExternalInput")
    Ident = nc.dram_tensor("ident", [128, 128], MDT, kind="ExternalInput")
    Mask01 = nc.dram_tensor("mask01", [128, 128], MDT, kind="ExternalInput")
    Emat = nc.dram_tensor("emat", [H, C], MDT, kind="ExternalInput")
    Ones1 = nc.dram_tensor("ones1", [1, 128], MDT, kind="ExternalInput")
    OnesV = nc.dram_tensor("onesv", [128, H], MDT, kind="ExternalInput")
    Y = nc.dram_tensor("y", [S, C], F32, kind="ExternalOutput")

    from contextlib import ExitStack
    with ExitStack() as _es:
        tc = _es.enter_context(tile.TileContext(nc))
        _p = lambda **kw: _es.enter_context(tc.tile_pool(**kw))
        cst = _p(name="cst", bufs=1)
        ld = _p(name="ld", bufs=2)
        xpe = _p(name="xpe", bufs=2)
        xpeT_p = _p(name="xpeT", bufs=NK)
        wqk_p = _p(name="wqk", bufs=NK)
        wvo_p = _p(name="wvo", bufs=NK)
        qkT_p = _p(name="qkT", bufs=3)
        vx_p = _p(name="vx", bufs=NS)
        pT_p = _p(name="pT", bufs=2)
        pvT_p = _p(name="pvT", bufs=NK)
        ysb_p = _p(name="ysb", bufs=2)
        dstage_p = _p(name="dstage", bufs=2)
        ps1 = _p(name="ps1", bufs=4, space="PSUM")    # 4 x 1-bank slots
        scps = _p(name="scps", bufs=2, space="PSUM")  # 2 x 2-bank slots

        def go():
            ident_sb = cst.tile([128, 128], MDT, tag="ident", name="ident_sb")
            nc.sync.dma_start(ident_sb[:], Ident.ap())
            mask_sb = cst.tile([128, 128], MDT, tag="mask", name="mask_sb")
            nc.sync.dma_start(mask_sb[:], Mask01.ap())
            bqk_sb = cst.tile([128, NM], F32, tag="bqk", name="bqk_sb")
            nc.sync.dma_start(bqk_sb[:], Bqk.ap())
            beff_sb = cst.tile([1, C], MDT, tag="beff", name="beff_sb")
            nc.sync.dma_start(beff_sb[:], Beff.ap())
            emat_sb = cst.tile([H, C], MDT, tag="emat", name="emat_sb")
            nc.sync.dma_start(emat_sb[:], Emat.ap())
            ones_sb = cst.tile([1, 128], MDT, tag="ones", name="ones_sb")
            nc.sync.dma_start(ones_sb[:], Ones1.ap())
            den_sb = cst.tile([H, S], F32, tag="den", name="den_sb")
            rden_sb = cst.tile([H, S], MDT, tag="rden", name="rden_sb")

            # Weights (wvo pool serves W_v first, then W_o reuses the slots)
            wqk_t = []
            for k in range(NK):
                w = wqk_p.tile([128, 2 * C], MDT, tag="wqk", name=f"wqk{k}")
                nc.sync.dma_start(w[:], Wqk.ap()[k * 128:(k + 1) * 128, :])
                wqk_t.append(w)
            wv_t = []
            for k in range(NK):
                w = wvo_p.tile([128, C], MDT, tag="wvo", name=f"wv{k}")
                nc.sync.dma_start(w[:], Wv.ap()[k * 128:(k + 1) * 128, :])
                wv_t.append(w)

            # ---- Phase A: xpeT = (x + pe)^T  [C, S] as NK tiles [128, S]
            xpeT = [xpeT_p.tile([128, S], MDT, tag="xpeT", name=f"xpeT{j}")
                    for j in range(NK)]
            for i in range(NS):
                xt = ld.tile([128, C], F32, tag="ldx", name=f"xt{i}")
                pt_ = ld.tile([128, C], F32, tag="ldp", name=f"pt{i}")
                nc.sync.dma_start(xt[:], Xb.ap()[i * 128:(i + 1) * 128, :])
                nc.sync.dma_start(pt_[:], PEb.ap()[i * 128:(i + 1) * 128, :])
                xp = xpe.tile([128, C], MDT, tag="xpe", name=f"xp{i}")
                nc.vector.tensor_tensor(xp[:], xt[:], pt_[:], ALU.add)
                for j in range(NK):
                    tp = ps1.tile([128, 128], MDT, tag="ps1", name=f"tp{i}_{j}")
                    nc.tensor.transpose(tp[:], xp[:, j * 128:(j + 1) * 128],
                                        ident_sb[:])
                    nc.any.tensor_copy(xpeT[j][:, i * 128:(i + 1) * 128], tp[:])

            # ---- Phase A2: V with ones column per head: vx[Ti] [128, 780]
            vx = []
            for si in range(NS):
                v = vx_p.tile([128, H * (CC + 1)], MDT, tag="vx", name=f"vx{si}")
                v3 = v[:].rearrange("p (h c) -> p h c", h=H)
                for (c0, c1) in ((0, 512), (512, C)):
                    vp = ps1.tile([128, c1 - c0], F32, tag="ps1", name=f"vp{si}_{c0}")
                    for k in range(NK):
                        nc.tensor.matmul(
                            vp[:], xpeT[k][:, si * 128:(si + 1) * 128],
                            wv_t[k][:, c0:c1],
                            start=(k == 0), stop=(k == NK - 1),
                        )
                    h0, h1 = c0 // CC, c1 // CC
                    nc.any.tensor_copy(
                        v3[:, h0:h1, 0:CC],
                        vp[:].rearrange("p (h c) -> p h c", h=h1 - h0),
                    )
                nc.sync.dma_start(v3[:, :, CC:CC + 1], OnesV.ap()[:, :, None])
                vx.append(v)

            # ---- Phases B+C: per pass p: qkT tiles (p, 6+p), heads 2p, 2p+1
            pvT = [pvT_p.tile([128, S], MDT, tag="pvT", name=f"pvT{j}")
                   for j in range(NK)]
            for p in range(6):
                qk = {}
                for mm in (p, 6 + p):
                    t = qkT_p.tile([128, S], MDT, tag="qkT", name=f"qkT{mm}")
                    for n2 in (0, 1):
                        ps_ = ps1.tile([128, 512], F32, tag="ps1",
                                       name=f"qkps{mm}_{n2}")
                        for k in range(NK):
                            nc.tensor.matmul(
                                ps_[:], wqk_t[k][:, mm * 128:(mm + 1) * 128],
                                xpeT[k][:, n2 * 512:(n2 + 1) * 512],
                                start=(k == 0), stop=(k == NK - 1),
                            )
                        nc.any.tensor_scalar_add(
                            t[:, n2 * 512:(n2 + 1) * 512], ps_[:],
                            bqk_sb[:, mm:mm + 1],
                        )
                    qk[mm] = t
                qt, kt = qk[p], qk[6 + p]
                heads = (2 * p, 2 * p + 1)
                pv = {(h, half): ps1.tile([CC + 1, 512], F32, tag="ps1",
                                          name=f"pv{h}_{half}")
                      for h in heads for half in (0, 1)}

                def evict(h, half):
                    r0 = (h % 2) * CC
                    c0 = half * 512
                    t_ = pv[(h, half)]
                    dst = dstage_p.tile([1, 512], F32, tag="dstage",
                                        name=f"dst{h}_{half}")
                    nc.any.tensor_copy(dst[:], t_[CC:CC + 1, :])
                    nc.sync.dma_start(den_sb[h:h + 1, c0:c0 + 512], dst[:])
                    nc.any.tensor_copy(pvT[p][r0:r0 + CC, c0:c0 + 512],
                                       t_[0:CC, :])
                for Ti in range(NS):
                    s0 = Ti * 128
                    slen = S - s0
                    chunks = [(s0, 512), (512, S)] if s0 < 512 else [(s0, S)]
                    # pt holds both heads: cols [0:slen]=head A, [slen:2*slen]=B
                    pt = pT_p.tile([128, 2 * slen], MDT, tag="pT",
                                   name=f"pt{p}_{Ti}")
                    off = {heads[0]: 0, heads[1]: slen}
                    for (a0, a1) in chunks:
                        w = a1 - a0
                        # head A at col 0, head B anchored at the 512 bank line
                        sc = scps.tile([128, 1024], F32, tag="scps",
                                       name=f"sc{p}_{Ti}_{a0}")
                        for hi, h in enumerate(heads):
                            r0 = (h % 2) * CC
                            nc.tensor.matmul(
                                sc[:, hi * 512:hi * 512 + w],
                                kt[r0:r0 + CC, s0:s0 + 128],
                                qt[r0:r0 + CC, a0:a1],
                                start=True, stop=True,
                            )
                        # one exp over both heads' chunks (strided 3D APs)
                        ap_out = pt[:].rearrange(
                            "q (i c) -> q i c", i=2)[:, :, a0 - s0:a1 - s0]
                        ap_in = sc[:].rearrange(
                            "q (i c) -> q i c", i=2)[:, :, 0:w]
                        nc.scalar.activation(
                            ap_out, ap_in,
                            AF.Exp, scale=float(1.0 / np.sqrt(CC)),
                        )
                        if a0 == s0:  # diagonal blocks: zero masked probs
                            for h in heads:
                                o = off[h]
                                nc.gpsimd.tensor_tensor(
                                    pt[:, o:o + 128], pt[:, o:o + 128],
                                    mask_sb[:], ALU.mult,
                                )
                    for h in heads:
                        v3 = vx[Ti][:].rearrange("p (h c) -> p h c", h=H)
                        o = off[h]
                        for (a0, a1) in chunks:
                            half = 0 if a1 <= 512 else 1
                            last_ti = 3 if half == 0 else NS - 1
                            c0 = half * 512
                            nc.tensor.matmul(
                                pv[(h, half)][:, a0 - c0:a1 - c0], v3[:, h, :],
                                pt[:, o + a0 - s0:o + a1 - s0],
                                start=(Ti == 0), stop=(Ti == last_ti),
                            )
                    if Ti == 3:
                        for h in heads:
                            evict(h, 0)
                for h in heads:
                    evict(h, 1)

            # ---- Phase D: normalize + output projection
            with nc.allow_low_precision(reason="fp32r rounding of softmax scale"):
                nc.vector.reciprocal(rden_sb[:], den_sb[:])
            for j in range(NK):
                for n2 in (0, 1):
                    bc = ps1.tile([128, 512], F32, tag="ps1", name=f"bc{j}_{n2}")
                    nc.tensor.matmul(
                        bc[:], emat_sb[:, j * 128:(j + 1) * 128],
                        rden_sb[:, n2 * 512:(n2 + 1) * 512],
                        start=True, stop=True,
                    )
                    nc.vector.tensor_tensor(
                        pvT[j][:, n2 * 512:(n2 + 1) * 512],
                        pvT[j][:, n2 * 512:(n2 + 1) * 512], bc[:], ALU.mult,
                    )

            wo_t = []
            for k in range(NK):
                w = wvo_p.tile([128, C], MDT, tag="wvo", name=f"wo{k}")
                nc.sync.dma_start(w[:], Wo.ap()[k * 128:(k + 1) * 128, :])
                wo_t.append(w)
            for si in range(NS):
                ty = ysb_p.tile([128, C], F32, tag="ysb", name=f"ty{si}")
                for (c0, c1) in ((0, 512), (512, C)):
                    yp = ps1.tile([128, c1 - c0], F32, tag="ps1",
                                  name=f"yp{si}_{c0}")
                    for k in range(NK):
                        nc.tensor.matmul(
                            yp[:], pvT[k][:, si * 128:(si + 1) * 128],
                            wo_t[k][:, c0:c1],
                            start=(k == 0), stop=False,
                        )
                    nc.tensor.matmul(
                        yp[:], ones_sb[:], beff_sb[:, c0:c1],
                        start=False, stop=True,
                    )
                    nc.any.tensor_copy(ty[:, c0:c1], yp[:])
                nc.sync.dma_start(Y.ap()[si * 128:(si + 1) * 128, :], ty[:])

        for _rep in range(REPEAT):
            go()

    nc.compile()
    return nc


def _round_fp22(a):
    """Round to nearest fp22 (11-bit mantissa) so the PE's truncation is exact."""
    b = a.astype(np.float32).view(np.uint32)
    b = (b + 0x800) & np.uint32(0xFFFFF000)
    return b.view(np.float32)


def _prep(inputs):
    x = np.asarray(inputs["x"], np.float32)
    pe = np.asarray(inputs["pe"], np.float32)
    W_qkv = np.asarray(inputs["W_qkv"], np.float32)
    b_qkv = np.asarray(inputs["b_qkv"], np.float32)
    W_out = np.asarray(inputs["W_out"], np.float32)
    b_out = np.asarray(inputs["b_out"], np.float32)

    rnd = (lambda a: a) if USE_F32 else _round_fp22
    wqk = rnd(np.ascontiguousarray(W_qkv[:, :2 * C]))
    wv = rnd(np.ascontiguousarray(W_qkv[:, 2 * C:]))
    wo = rnd(np.ascontiguousarray(W_out))
    bqk = np.ascontiguousarray(b_qkv[:2 * C].reshape(NM, 128).T)
    beff = rnd((b_qkv[2 * C:] @ W_out + b_out).reshape(1, C).astype(np.float32))
    ident = np.eye(128, dtype=np.float32)
    t = np.arange(128)
    mask01 = (t[:, None] <= t[None, :]).astype(np.float32)
    emat = np.zeros((H, C), np.float32)
    for h in range(H):
        emat[h, h * CC:(h + 1) * CC] = 1.0
    ones1 = np.ones((1, 128), np.float32)

    common = dict(wqk=wqk, wv=wv, wo=wo, bqk=bqk, beff=beff, ident=ident,
                  mask01=mask01, emat=emat, ones1=ones1,
                  onesv=np.ones((128, H), np.float32))
    in_maps = []
    for b in range(B):
        m = dict(common)
        m["xb"] = np.ascontiguousarray(x[:, b, :])
        m["peb"] = np.ascontiguousarray(pe[:, b, :])
        in_maps.append(m)
    return in_maps


def _run(inputs, trace=False):
    if "nc" not in _CACHE:
        _CACHE["nc"] = _build()
    nc = _CACHE["nc"]
    in_maps = _prep(inputs)
    res = run_bass_kernel_spmd(nc, in_maps, core_ids=list(range(B)), trace=trace)
    out = np.empty((S, B, C), np.float32)
    for b in range(B):
        out[:, b, :] = res.results[b]["y"]
    return out, res


def kernel(**inputs):
    out, _ = _run(inputs, trace=False)
    return out


# revision 20
# speedup vs baseline: 614.7083x; 1.2299x over previous
"""TRN2 Bass kernel for CompressedCausalAttention.

Problem: x,pe [S=1024, B=8, C=768]; qkv = (x+pe) @ W_qkv + b_qkv; 12 heads of 64;
causal softmax attention; out proj. Sharding: data-parallel over batch, one
batch element per NeuronCore (8 cores).

Per-core dataflow (channel-major to avoid transposing activations twice):
  xpeT [C, S]   = PE-transpose(x+pe)
  qkT  [2C', S] = W_qk^T @ xpeT (+ b_qk per-partition during PSUM->SBUF copy)
  V    [S, C]   = xpeT^T @ W_v  (stored per t-block with a ones column per head
                                 -> PV matmul also produces softmax denominators)
  per head h, per t-block Ti (both heads of a pass interleaved so their K=64
  QK matmuls occupy disjoint PE row halves and run concurrently):
     scoresT[t, s] = kT_h[:,Ti]^T @ qT_h   (only s >= 128*Ti; causal)
     pT = exp(0.125 * scoresT)             (ACT, PSUM->SBUF)
     diag block *= upper-tri 0/1 mask      (GPSIMD; zero masked probs)
     pv[cc+1, s] += V'_h[Ti]^T @ pT        (row 64 = denominator)
  normalize: rdenom = 1/denom; broadcast over head rows via 0/1 E-matrix matmul;
  pvT *= bcast.  y [S, C] = pvT^T @ W_o + b_out_eff (rank-1 ones matmul).
  b_out_eff folds the V bias through W_out on the host (softmax rows sum to 1).

Matmuls run in fp32r (fp32 truncated to ~11-bit mantissa at the PE, fp32
accumulate) for 4x PE throughput vs true fp32; set BASSK_F32=1 for full fp32.
"""
import os
import numpy as np

import concourse.bass as bass
import concourse.bacc as bacc
import concourse.mybir as mybir
import concourse.tile as tile
from concourse.bass_utils import run_bass_kernel_spmd

S, B, C, H = 1024, 8, 768, 12
CC = C // H            # 64
NS = S // 128          # 8 s/t blocks
NK = C // 128          # 6 contraction chunks
NM = 2 * C // 128      # 12 q+k M-tiles
F32 = mybir.dt.float32
USE_F32 = os.environ.get("BASSK_F32", "0") == "1"
REPEAT = int(os.environ.get("BASSK_REPEAT", "1"))
MDT = F32 if USE_F32 else mybir.dt.float32r
AF = mybir.ActivationFunctionType
ALU = mybir.AluOpType

_CACHE = {}


def _build():
    nc = bacc.Bacc("TRN2", target_bir_lowering=False, debug=False)

    Xb = nc.dram_tensor("xb", [S, C], F32, kind="ExternalInput")
    PEb = nc.dram_tensor("peb", [S, C], F32, kind="ExternalInput")
    Wqk = nc.dram_tensor("wqk", [C, 2 * C], MDT, kind="ExternalInput")
    Wv = nc.dram_tensor("wv", [C, C], MDT, kind="ExternalInput")
    Wo = nc.dram_tensor("wo", [C, C], MDT, kind="ExternalInput")
    Bqk = nc.dram_tensor("bqk", [128, NM], F32, kind="ExternalInput")
    Beff = nc.dram_tensor("beff", [1, C], MDT, kind="ExternalInput")
    Ident = nc.dram_tensor("ident", [128, 128], MDT, kind="ExternalInput")
    Mask01 = nc.dram_tensor("mask01", [128, 128], MDT, kind="ExternalInput")
    Emat = nc.dram_tensor("emat", [H, C], MDT, kind="ExternalInput")
    Ones1 = nc.dram_tensor("ones1", [1, 128], MDT, kind="ExternalInput")
    OnesV = nc.dram_tensor("onesv", [128, H], MDT, kind="ExternalInput")
    Y = nc.dram_tensor("y", [S, C], F32, kind="ExternalOutput")

    from contextlib import ExitStack
    with ExitStack() as _es:
        tc = _es.enter_context(tile.TileContext(nc))
        _p = lambda **kw: _es.enter_context(tc.tile_pool(**kw))
        cst = _p(name="cst", bufs=1)
        ld = _p(name="ld", bufs=2)
        xpe = _p(name="xpe", bufs=2)
        xpeT_p = _p(name="xpeT", bufs=NK)
        wqk_p = _p(name="wqk", bufs=NK)
        wvo_p = _p(name="wvo", bufs=NK)
        qkT_p = _p(name="qkT", bufs=5)
        vx_p = _p(name="vx", bufs=NS)
        pT_p = _p(name="pT", bufs=2)
        pvT_p = _p(name="pvT", bufs=NK)
        ysb_p = _p(name="ysb", bufs=2)
        dstage_p = _p(name="dstage", bufs=2)
        ps1 = _p(name="ps1", bufs=4, space="PSUM")    # 4 x 1-bank slots
        scps = _p(name="scps", bufs=2, space="PSUM")  # 2 x 2-bank slots

        def go():
            ident_sb = cst.tile([128, 128], MDT, tag="ident", name="ident_sb")
            nc.sync.dma_start(ident_sb[:], Ident.ap())
            mask_sb = cst.tile([128, 128], MDT, tag="mask", name="mask_sb")
            nc.sync.dma_start(mask_sb[:], Mask01.ap())
            bqk_sb = cst.tile([128, NM], F32, tag="bqk", name="bqk_sb")
            nc.sync.dma_start(bqk_sb[:], Bqk.ap())
            beff_sb = cst.tile([1, C], MDT, tag="beff", name="beff_sb")
            nc.sync.dma_start(beff_sb[:], Beff.ap())
            emat_sb = cst.tile([H, C], MDT, tag="emat", name="emat_sb")
            nc.sync.dma_start(emat_sb[:], Emat.ap())
            ones_sb = cst.tile([1, 128], MDT, tag="ones", name="ones_sb")
            nc.sync.dma_start(ones_sb[:], Ones1.ap())
            den_sb = cst.tile([H, S], F32, tag="den", name="den_sb")
            rden_sb = cst.tile([H, S], MDT, tag="rden", name="rden_sb")

            # ---- Phase A: xpeT = (x + pe)^T  [C, S] as NK tiles [128, S]
            xpeT = [xpeT_p.tile([128, S], MDT, tag="xpeT", name=f"xpeT{j}")
                    for j in range(NK)]
            for i in range(NS):
                xt = ld.tile([128, C], F32, tag="ldx", name=f"xt{i}")
                pt_ = ld.tile([128, C], F32, tag="ldp", name=f"pt{i}")
                nc.sync.dma_start(xt[:], Xb.ap()[i * 128:(i + 1) * 128, :])
                nc.sync.dma_start(pt_[:], PEb.ap()[i * 128:(i + 1) * 128, :])
                xp = xpe.tile([128, C], MDT, tag="xpe", name=f"xp{i}")
                nc.vector.tensor_tensor(xp[:], xt[:], pt_[:], ALU.add)
                for j in range(NK):
                    tp = ps1.tile([128, 128], MDT, tag="ps1", name=f"tp{i}_{j}")
                    nc.tensor.transpose(tp[:], xp[:, j * 128:(j + 1) * 128],
                                        ident_sb[:])
                    nc.any.tensor_copy(xpeT[j][:, i * 128:(i + 1) * 128], tp[:])

            # W_v loads (queued after x/pe so phase A starts immediately)
            wv_t = []
            for k in range(NK):
                w = wvo_p.tile([128, C], MDT, tag="wvo", name=f"wv{k}")
                nc.sync.dma_start(w[:], Wv.ap()[k * 128:(k + 1) * 128, :])
                wv_t.append(w)

            # ---- Phase A2: V with ones column per head: vx[Ti] [128, 780]
            vx = []
            for si in range(NS):
                v = vx_p.tile([128, H * (CC + 1)], MDT, tag="vx", name=f"vx{si}")
                v3 = v[:].rearrange("p (h c) -> p h c", h=H)
                for (c0, c1) in ((0, 512), (512, C)):
                    vp = ps1.tile([128, c1 - c0], F32, tag="ps1", name=f"vp{si}_{c0}")
                    for k in range(NK):
                        nc.tensor.matmul(
                            vp[:], xpeT[k][:, si * 128:(si + 1) * 128],
                            wv_t[k][:, c0:c1],
                            start=(k == 0), stop=(k == NK - 1),
                        )
                    h0, h1 = c0 // CC, c1 // CC
                    nc.vector.tensor_copy(
                        v3[:, h0:h1, 0:CC],
                        vp[:].rearrange("p (h c) -> p h c", h=h1 - h0),
                    )
                nc.sync.dma_start(v3[:, :, CC:CC + 1], OnesV.ap()[:, :, None])
                vx.append(v)

            wqk_t = []
            for k in range(NK):
                w = wqk_p.tile([128, 2 * C], MDT, tag="wqk", name=f"wqk{k}")
                nc.sync.dma_start(w[:], Wqk.ap()[k * 128:(k + 1) * 128, :])
                wqk_t.append(w)

            # ---- Phases B+C: per pass p: qkT tiles (p, 6+p), heads 2p, 2p+1
            pvT = [pvT_p.tile([128, S], MDT, tag="pvT", name=f"pvT{j}")
                   for j in range(NK)]
            for p in range(6):
                qk = {}
                for mm in (p, 6 + p):
                    t = qkT_p.tile([128, S], MDT, tag="qkT", name=f"qkT{mm}")
                    for n2 in (0, 1):
                        ps_ = ps1.tile([128, 512], F32, tag="ps1",
                                       name=f"qkps{mm}_{n2}")
                        for k in range(NK):
                            nc.tensor.matmul(
                                ps_[:], wqk_t[k][:, mm * 128:(mm + 1) * 128],
                                xpeT[k][:, n2 * 512:(n2 + 1) * 512],
                                start=(k == 0), stop=(k == NK - 1),
                            )
                        nc.any.tensor_scalar_add(
                            t[:, n2 * 512:(n2 + 1) * 512], ps_[:],
                            bqk_sb[:, mm:mm + 1],
                        )
                    qk[mm] = t
                qt, kt = qk[p], qk[6 + p]
                heads = (2 * p, 2 * p + 1)
                pv = {(h, half): ps1.tile([CC + 1, 512], F32, tag="ps1",
                                          name=f"pv{h}_{half}")
                      for h in heads for half in (0, 1)}

                def evict(h, half):
                    r0 = (h % 2) * CC
                    c0 = half * 512
                    t_ = pv[(h, half)]
                    dst = dstage_p.tile([1, 512], F32, tag="dstage",
                                        name=f"dst{h}_{half}")
                    nc.vector.tensor_copy(dst[:], t_[CC:CC + 1, :])
                    nc.sync.dma_start(den_sb[h:h + 1, c0:c0 + 512], dst[:])
                    nc.vector.tensor_copy(pvT[p][r0:r0 + CC, c0:c0 + 512],
                                       t_[0:CC, :])
                for Ti in range(NS):
                    s0 = Ti * 128
                    slen = S - s0
                    chunks = [(s0, 512), (512, S)] if s0 < 512 else [(s0, S)]
                    # pt holds both heads: cols [0:slen]=head A, [slen:2*slen]=B
                    pt = pT_p.tile([128, 2 * slen], MDT, tag="pT",
                                   name=f"pt{p}_{Ti}")
                    off = {heads[0]: 0, heads[1]: slen}
                    for (a0, a1) in chunks:
                        w = a1 - a0
                        # head A at col 0, head B anchored at the 512 bank line
                        sc = scps.tile([128, 1024], F32, tag="scps",
                                       name=f"sc{p}_{Ti}_{a0}")
                        for hi, h in enumerate(heads):
                            r0 = (h % 2) * CC
                            nc.tensor.matmul(
                                sc[:, hi * 512:hi * 512 + w],
                                kt[r0:r0 + CC, s0:s0 + 128],
                                qt[r0:r0 + CC, a0:a1],
                                start=True, stop=True,
                            )
                        # one exp over both heads' chunks (strided 3D APs)
                        ap_out = pt[:].rearrange(
                            "q (i c) -> q i c", i=2)[:, :, a0 - s0:a1 - s0]
                        ap_in = sc[:].rearrange(
                            "q (i c) -> q i c", i=2)[:, :, 0:w]
                        nc.scalar.activation(
                            ap_out, ap_in,
                            AF.Exp, scale=float(1.0 / np.sqrt(CC)),
                        )
                        if a0 == s0:  # diagonal blocks: zero masked probs
                            for h in heads:
                                o = off[h]
                                nc.gpsimd.tensor_tensor(
                                    pt[:, o:o + 128], pt[:, o:o + 128],
                                    mask_sb[:], ALU.mult,
                                )
                    for h in heads:
                        v3 = vx[Ti][:].rearrange("p (h c) -> p h c", h=H)
                        o = off[h]
                        for (a0, a1) in chunks:
                            half = 0 if a1 <= 512 else 1
                            last_ti = 3 if half == 0 else NS - 1
                            c0 = half * 512
                            nc.tensor.matmul(
                                pv[(h, half)][:, a0 - c0:a1 - c0], v3[:, h, :],
                                pt[:, o + a0 - s0:o + a1 - s0],
                                start=(Ti == 0), stop=(Ti == last_ti),
                            )
                    if Ti == 3:
                        for h in heads:
                            evict(h, 0)
                for h in heads:
                    evict(h, 1)

            # ---- Phase D: normalize + output projection
            with nc.allow_low_precision(reason="fp32r rounding of softmax scale"):
                nc.vector.reciprocal(rden_sb[:], den_sb[:])
            for j in range(NK):
                for n2 in (0, 1):
                    bc = ps1.tile([128, 512], F32, tag="ps1", name=f"bc{j}_{n2}")
                    nc.tensor.matmul(
                        bc[:], emat_sb[:, j * 128:(j + 1) * 128],
                        rden_sb[:, n2 * 512:(n2 + 1) * 512],
                        start=True, stop=True,
                    )
                    nc.vector.tensor_tensor(
                        pvT[j][:, n2 * 512:(n2 + 1) * 512],
                        pvT[j][:, n2 * 512:(n2 + 1) * 512], bc[:], ALU.mult,
                    )

            wo_t = []
            for k in range(NK):
                w = wvo_p.tile([128, C], MDT, tag="wvo", name=f"wo{k}")
                nc.sync.dma_start(w[:], Wo.ap()[k * 128:(k + 1) * 128, :])
                wo_t.append(w)
            for si in range(NS):
                ty = ysb_p.tile([128, C], F32, tag="ysb", name=f"ty{si}")
                for (c0, c1) in ((0, 512), (512, C)):
                    yp = ps1.tile([128, c1 - c0], F32, tag="ps1",
                                  name=f"yp{si}_{c0}")
                    for k in range(NK):
                        nc.tensor.matmul(
                            yp[:], pvT[k][:, si * 128:(si + 1) * 128],
                            wo_t[k][:, c0:c1],
                            start=(k == 0), stop=False,
                        )
                    nc.tensor.matmul(
                        yp[:], ones_sb[:], beff_sb[:, c0:c1],
                        start=False, stop=True,
                    )
                    nc.vector.tensor_copy(ty[:, c0:c1], yp[:])
                nc.sync.dma_start(Y.ap()[si * 128:(si + 1) * 128, :], ty[:])

        for _rep in range(REPEAT):
            go()

    nc.compile()
    return nc


def _round_fp22(a):
    """Round to nearest fp22 (11-bit mantissa) so the PE's truncation is exact."""
    b = a.astype(np.float32).view(np.uint32)
    b = (b + 0x800) & np.uint32(0xFFFFF000)
    return b.view(np.float32)


def _prep(inputs):
    x = np.asarray(inputs["x"], np.float32)
    pe = np.asarray(inputs["pe"], np.float32)
    W_qkv = np.asarray(inputs["W_qkv"], np.float32)
    b_qkv = np.asarray(inputs["b_qkv"], np.float32)
    W_out = np.asarray(inputs["W_out"], np.float32)
    b_out = np.asarray(inputs["b_out"], np.float32)

    rnd = (lambda a: a) if USE_F32 else _round_fp22
    wqk = rnd(np.ascontiguousarray(W_qkv[:, :2 * C]))
    wv = rnd(np.ascontiguousarray(W_qkv[:, 2 * C:]))
    wo = rnd(np.ascontiguousarray(W_out))
    bqk = np.ascontiguousarray(b_qkv[:2 * C].reshape(NM, 128).T)
    beff = rnd((b_qkv[2 * C:] @ W_out + b_out).reshape(1, C).astype(np.float32))
    ident = np.eye(128, dtype=np.float32)
    t = np.arange(128)
    mask01 = (t[:, None] <= t[None, :]).astype(np.float32)
    emat = np.zeros((H, C), np.float32)
    for h in range(H):
        emat[h, h * CC:(h + 1) * CC] = 1.0
    ones1 = np.ones((1, 128), np.float32)

    common = dict(wqk=wqk, wv=wv, wo=wo, bqk=bqk, beff=beff, ident=ident,
                  mask01=mask01, emat=emat, ones1=ones1,
                  onesv=np.ones((128, H), np.float32))
    in_maps = []
    for b in range(B):
        m = dict(common)
        m["xb"] = np.ascontiguousarray(x[:, b, :])
        m["peb"] = np.ascontiguousarray(pe[:, b, :])
        in_maps.append(m)
    return in_maps


def _run(inputs, trace=False):
    if "nc" not in _CACHE:
        _CACHE["nc"] = _build()
    nc = _CACHE["nc"]
    in_maps = _prep(inputs)
    res = run_bass_kernel_spmd(nc, in_maps, core_ids=list(range(B)), trace=trace)
    out = np.empty((S, B, C), np.float32)
    for b in range(B):
        out[:, b, :] = res.results[b]["y"]
    return out, res


def kernel(**inputs):
    out, _ = _run(inputs, trace=False)
    return out
